# revision 2
# baseline (speedup 1.0000x reference)
"""Single-head attention (B=4, S=2048, E=1024) on 8 TRN2 NeuronCores.

Sharding: data-parallel over (batch, query-half): core c handles batch c//2,
queries [h*1024, (h+1)*1024) where h = c%2. The host permutes the key/value
sequence so this core's query block is always columns [0, 1024) of xt
(attention output is invariant to a consistent permutation of keys+values).

Algebraic reassociation removes the per-core Q/K/V projections entirely:
  scores = (x_q M + r) x^T   with M = Wq^T Wk / sqrt(E), r = bq Wk / sqrt(E)
           (host weight-only precompute; bk terms are softmax-invariant)
  out    = (attn x) Wv^T + bv
Phases (bf16 matmuls, fp32 PSUM):
  1 QM^T[eo,q] = M^T-blocks @ xt_q (+r)      8 groups x 8 MM (1024-wide)
  2 scoresT[k,q] = xt-blocks @ QM^T, exp     16 groups x 8 MM; DVE
    accumulates rowsum partials in the shadow
  3 TT[e,q] = x-blocks @ attnT               8 groups x 8 MM
  rs rsT[q,1] per q-block = acc^T ones       8 one-column fp32 MMs
  4 out[q,eo] = TT^T Wv^T * (1/rowsum) + bv  16 groups x 8 MM (512-wide)
"""
import numpy as np
import ml_dtypes

import concourse.bass as bass
import concourse.bacc as bacc
import concourse.mybir as mybir
from concourse.tile import TileContext
from concourse.bass_utils import run_bass_kernel_spmd
from concourse.masks import make_identity

B, S, E = 4, 2048, 1024
P = 128
EC = E // P          # 8 e-chunks (contraction)
SKC = S // P         # 16 key chunks
SQ = S // 2          # queries per core
QB = SQ // P         # 8 query blocks per core
NG = 512
INV_SCALE = 1.0 / float(np.sqrt(E))

MM_DT = mybir.dt.bfloat16
NP_MM = ml_dtypes.bfloat16
F32 = mybir.dt.float32

_CACHE = {}


def _build():
    nc = bacc.Bacc()
    xq2 = nc.declare_dram_parameter("xq2", [P, EC * SQ], MM_DT, isOutput=False)
    xk2 = nc.declare_dram_parameter("xk2", [P, EC * SQ], MM_DT, isOutput=False)
    xn2 = nc.declare_dram_parameter("xn2", [P, SKC * E], MM_DT, isOutput=False)
    mc2 = nc.declare_dram_parameter("mc2", [P, EC * E], MM_DT, isOutput=False)
    wvt2 = nc.declare_dram_parameter("wvt2", [P, EC * E], MM_DT, isOutput=False)
    rr = nc.declare_dram_parameter("rr", [P, EC], F32, isOutput=False)
    bvb = nc.declare_dram_parameter("bvb", [P, E], F32, isOutput=False)
    out = nc.declare_dram_parameter("out", [SQ, E], MM_DT, isOutput=True)

    xq_r = xq2[:, :].rearrange("p (ec s) -> p ec s", ec=EC)
    xk_r = xk2[:, :].rearrange("p (ec s) -> p ec s", ec=EC)
    xn_r = xn2[:, :].rearrange("p (kc e) -> p kc e", kc=SKC)
    mc_r = mc2[:, :].rearrange("p (ec e) -> p ec e", ec=EC)
    wvt_r = wvt2[:, :].rearrange("p (ec e) -> p ec e", ec=EC)

    with TileContext(nc) as tc:
        with (
            tc.tile_pool(name="wp", bufs=1) as wp,
            tc.tile_pool(name="kvq", bufs=1) as kvq,
            tc.tile_pool(name="outp", bufs=2) as outp,
            tc.tile_pool(name="smalls", bufs=2) as smalls,
            tc.tile_pool(name="ps", bufs=4, space="PSUM") as ps,
            tc.tile_pool(name="pw", bufs=2, space="PSUM") as pw,
        ):
            ident = wp.tile([P, P], MM_DT)
            make_identity(nc, ident)
            ones_f = wp.tile([P, 1], F32)
            nc.gpsimd.memset(ones_f, 1.0)

            mc_sb = wp.tile([P, EC, E], MM_DT)
            xq_sb = wp.tile([P, EC, SQ], MM_DT)
            xk_sb = wp.tile([P, EC, SQ], MM_DT)
            xn_sb = wp.tile([P, SKC, E], MM_DT)
            wv_sb = wp.tile([P, EC, E], MM_DT)

            # ---- loads: phase-1-critical 4MB split across the two HWDGE
            # queues (mc on sync, xt-q on scalar). The scalar queue gets
            # ONLY those 8 issues: later ACTIVATEs queue behind DMA issues
            # (strict FIFO), so any extra scalar DMA stalls phase 1.
            rr_sb = wp.tile([P, EC], F32)
            bv_sb = wp.tile([P, E], F32)
            nc.sync.dma_start(rr_sb[:], rr[:, :])
            half = EC // 2
            nc.sync.dma_start(mc_sb[:, 0:half, :], mc_r[:, 0:half, :])
            nc.sync.dma_start(xq_sb[:, 0:half, :], xq_r[:, 0:half, :])
            nc.sync.dma_start(mc_sb[:, half:EC, :], mc_r[:, half:EC, :])
            nc.sync.dma_start(xq_sb[:, half:EC, :], xq_r[:, half:EC, :])
            nc.scalar.dma_start(bv_sb[:], bvb[:, :])
            nc.sync.dma_start(xk_sb[:, :, :], xk_r[:, :, :])
            nc.sync.dma_start(xn_sb[:, :, :], xn_r[:, :, :])
            nc.sync.dma_start(wv_sb[:, :, :], wvt_r[:, :, :])

            # PE warmup: cover the initial DMA latency and release the HAM
            # clock throttle before real matmuls arrive.
            warm_ps = pw.tile([P, P], MM_DT, tag="warm", bufs=2)
            for _ in range(56):
                nc.tensor.transpose(warm_ps[:], ident[:], ident[:])

            QT = kvq.tile([P, EC, SQ], MM_DT)
            attnT = kvq.tile([P, SKC, SQ], MM_DT)
            TT = kvq.tile([P, EC, SQ], MM_DT)

            # ---- phase 1: QM^T[eo, q] = (x_q M)^T + r ----
            for eo in range(EC):
                for g in range(2):
                    gsl = slice(g * NG, (g + 1) * NG)
                    pq = ps.tile([P, NG], F32, tag="mm")
                    for ec in range(EC):
                        nc.tensor.matmul(
                            pq[:], mc_sb[:, ec, eo * P:(eo + 1) * P],
                            xq_sb[:, ec, gsl],
                            start=(ec == 0), stop=(ec == EC - 1),
                        )
                    nc.scalar.activation(
                        QT[:, eo, gsl], pq[:],
                        mybir.ActivationFunctionType.Identity,
                        bias=rr_sb[:, eo:eo + 1],
                    )

            # ---- phase 2: scoresT[k, q] = xt_k^T QM^T ; attnT = exp ----
            # DVE accumulates rowsum partials (acc[p,q] = sum_kc attnT)
            acc = kvq.tile([P, SQ], F32)
            for kb in range(SKC):
                xsb, kbi = (xq_sb, kb) if kb < QB else (xk_sb, kb - QB)
                for g in range(2):
                    gsl = slice(g * NG, (g + 1) * NG)
                    p0 = ps.tile([P, NG], F32, tag="mm")
                    for ec in range(EC):
                        nc.tensor.matmul(
                            p0[:], xsb[:, ec, kbi * P:(kbi + 1) * P],
                            QT[:, ec, gsl],
                            start=(ec == 0), stop=(ec == EC - 1),
                        )
                    nc.scalar.activation(
                        attnT[:, kb, gsl], p0[:],
                        mybir.ActivationFunctionType.Exp,
                    )
                    if kb == 0:
                        nc.vector.tensor_copy(acc[:, gsl], attnT[:, 0, gsl])
                    else:
                        nc.vector.tensor_add(
                            acc[:, gsl], acc[:, gsl], attnT[:, kb, gsl]
                        )

            # ---- phase 3: TT[e, q] = x^T-blocks @ attnT ----
            for eb in range(EC):
                for g in range(2):
                    gsl = slice(g * NG, (g + 1) * NG)
                    t0 = ps.tile([P, NG], F32, tag="mm")
                    for kc in range(SKC):
                        nc.tensor.matmul(
                            t0[:], xn_sb[:, kc, eb * P:(eb + 1) * P],
                            attnT[:, kc, gsl],
                            start=(kc == 0), stop=(kc == SKC - 1),
                        )
                    nc.vector.tensor_copy(TT[:, eb, gsl], t0[:])

            # ---- rowsums: rsT[q,qb] via 8 one-column fp32 matmuls ----
            rsT = smalls.tile([P, QB], F32, tag="rsT")
            for qb in range(QB):
                pt = ps.tile([P, 1], F32, tag="mm")
                nc.tensor.matmul(
                    pt[:], acc[:, qb * P:(qb + 1) * P], ones_f[:, 0:1],
                    start=True, stop=True,
                )
                nc.vector.tensor_copy(rsT[:, qb:qb + 1], pt[:])
            recip = smalls.tile([P, QB], F32, tag="recip")
            nc.vector.reciprocal(recip[:], rsT[:])

            # ---- phase 4: out[q, eo] = TT^T Wv^T * recip + bv ----
            for qb in range(QB):
                outt = outp.tile([P, E], MM_DT, tag="out")
                for g in range(2):
                    gsl = slice(g * NG, (g + 1) * NG)
                    o0 = ps.tile([P, NG], F32, tag="mm")
                    for ec in range(EC):
                        nc.tensor.matmul(
                            o0[:], TT[:, ec, qb * P:(qb + 1) * P],
                            wv_sb[:, ec, gsl],
                            start=(ec == 0), stop=(ec == EC - 1),
                        )
                    nc.scalar.activation(
                        outt[:, gsl], o0[:],
                        mybir.ActivationFunctionType.Copy,
                        scale=recip[:, qb:qb + 1],
                    )
                    nc.vector.tensor_add(outt[:, gsl], outt[:, gsl], bv_sb[:, gsl])
                    nc.sync.dma_start(out[qb * P:(qb + 1) * P, gsl], outt[:, gsl])
    nc.finalize()
    return nc


def build_in_maps(x, Wq, bq, Wk, bk, Wv, bv):
    x = np.asarray(x, dtype=np.float32)
    Wq = np.asarray(Wq, np.float32)
    Wk = np.asarray(Wk, np.float32)
    Wv = np.asarray(Wv, np.float32)
    bq = np.asarray(bq, np.float32)
    bv = np.asarray(bv, np.float32)

    # weight-only precompute: fold Wq/Wk and the 1/sqrt(E) scale
    M = (Wq.T @ Wk) * np.float32(INV_SCALE)
    r = (bq @ Wk) * np.float32(INV_SCALE)
    def _chunked(a, n_chunks):
        f = a.shape[1]
        return np.ascontiguousarray(
            a.reshape(n_chunks, P, f).transpose(1, 0, 2).reshape(P, n_chunks * f)
        )

    mcm = _chunked(M.astype(NP_MM), EC)
    wvtm = _chunked(np.ascontiguousarray(Wv.T).astype(NP_MM), EC)
    rrm = np.ascontiguousarray(r.reshape(EC, P).T)      # [P, EC]
    bvb = np.broadcast_to(bv[None, :], (P, E)).copy()

    in_maps = []
    for c in range(8):
        b, h = divmod(c, 2)
        xb = x[b].astype(NP_MM)                         # [S, E]
        if h == 1:
            xb = np.concatenate([xb[SQ:], xb[:SQ]], axis=0)
        xnm = _chunked(xb, SKC)
        xtp = np.ascontiguousarray(xb.T)                # [E, S]
        xqm = _chunked(np.ascontiguousarray(xtp[:, 0:SQ]), EC)
        xkm = _chunked(np.ascontiguousarray(xtp[:, SQ:S]), EC)
        in_maps.append(
            dict(xq2=xqm, xk2=xkm, xn2=xnm, mc2=mcm, wvt2=wvtm, rr=rrm, bvb=bvb)
        )

    return in_maps


def kernel(x, Wq, bq, Wk, bk, Wv, bv):
    if "nc" not in _CACHE:
        _CACHE["nc"] = _build()
    nc = _CACHE["nc"]
    in_maps = build_in_maps(x, Wq, bq, Wk, bk, Wv, bv)
    res = run_bass_kernel_spmd(nc, in_maps, list(range(8)))

    out = np.empty((B, S, E), np.float32)
    for c in range(8):
        b, h = divmod(c, 2)
        out[b, h * SQ:(h + 1) * SQ, :] = np.asarray(res.results[c]["out"], np.float32)
    return out


# revision 3
# speedup vs baseline: 1.0356x; 1.0356x over previous
"""Single-head attention (B=4, S=2048, E=1024) on 8 TRN2 NeuronCores.

Sharding: data-parallel over (batch, query-half): core c handles batch c//2,
queries [h*1024, (h+1)*1024) where h = c%2. The host permutes the key/value
sequence so this core's query block is always columns [0, 1024) of xt
(attention output is invariant to a consistent permutation of keys+values).

Algebraic reassociation removes the per-core Q/K/V projections entirely:
  scores = (x_q M + r) x^T   with M = Wq^T Wk / sqrt(E), r = bq Wk / sqrt(E)
           (host weight-only precompute; bk terms are softmax-invariant)
  out    = (attn x) Wv^T + bv
Phases (bf16 matmuls, fp32 PSUM):
  1 QM^T[eo,q] = M^T-blocks @ xt_q (+r)      8 groups x 8 MM (1024-wide)
  2 scoresT[k,q] = xt-blocks @ QM^T, exp     16 groups x 8 MM; DVE
    accumulates rowsum partials in the shadow
  3 TT[e,q] = x-blocks @ attnT               8 groups x 8 MM
  rs rsT[q,1] per q-block = acc^T ones       8 one-column fp32 MMs
  4 out[q,eo] = TT^T Wv^T * (1/rowsum) + bv  16 groups x 8 MM (512-wide)
"""
import numpy as np
import ml_dtypes

import concourse.bass as bass
import concourse.bacc as bacc
import concourse.mybir as mybir
from concourse.tile import TileContext
from concourse.bass_utils import run_bass_kernel_spmd
from concourse.masks import make_identity

B, S, E = 4, 2048, 1024
P = 128
EC = E // P          # 8 e-chunks (contraction)
SKC = S // P         # 16 key chunks
SQ = S // 2          # queries per core
QB = SQ // P         # 8 query blocks per core
NG = 512
INV_SCALE = 1.0 / float(np.sqrt(E))

MM_DT = mybir.dt.bfloat16
NP_MM = ml_dtypes.bfloat16
F32 = mybir.dt.float32

_CACHE = {}


def _build():
    nc = bacc.Bacc()
    xq2 = nc.declare_dram_parameter("xq2", [P, EC * SQ], MM_DT, isOutput=False)
    xk2 = nc.declare_dram_parameter("xk2", [P, EC * SQ], MM_DT, isOutput=False)
    xn2 = nc.declare_dram_parameter("xn2", [P, SKC * E], MM_DT, isOutput=False)
    mc2 = nc.declare_dram_parameter("mc2", [P, EC * E], MM_DT, isOutput=False)
    wvt2 = nc.declare_dram_parameter("wvt2", [P, EC * E], MM_DT, isOutput=False)
    rr = nc.declare_dram_parameter("rr", [P, EC], F32, isOutput=False)
    bvb = nc.declare_dram_parameter("bvb", [P, E], F32, isOutput=False)
    out = nc.declare_dram_parameter("out", [SQ, E], MM_DT, isOutput=True)

    xq_r = xq2[:, :].rearrange("p (ec s) -> p ec s", ec=EC)
    xk_r = xk2[:, :].rearrange("p (ec s) -> p ec s", ec=EC)
    xn_r = xn2[:, :].rearrange("p (kc e) -> p kc e", kc=SKC)
    mc_r = mc2[:, :].rearrange("p (ec e) -> p ec e", ec=EC)
    wvt_r = wvt2[:, :].rearrange("p (ec e) -> p ec e", ec=EC)

    with TileContext(nc) as tc:
        with (
            tc.tile_pool(name="wp", bufs=1) as wp,
            tc.tile_pool(name="kvq", bufs=1) as kvq,
            tc.tile_pool(name="outp", bufs=2) as outp,
            tc.tile_pool(name="smalls", bufs=2) as smalls,
            tc.tile_pool(name="ps", bufs=4, space="PSUM") as ps,
            tc.tile_pool(name="pw", bufs=2, space="PSUM") as pw,
        ):
            ident = wp.tile([P, P], MM_DT)
            make_identity(nc, ident)
            ones_f = wp.tile([P, 1], F32)
            nc.gpsimd.memset(ones_f, 1.0)

            mc_sb = wp.tile([P, EC, E], MM_DT)
            xq_sb = wp.tile([P, EC, SQ], MM_DT)
            xk_sb = wp.tile([P, EC, SQ], MM_DT)
            xn_sb = wp.tile([P, SKC, E], MM_DT)
            wv_sb = wp.tile([P, EC, E], MM_DT)

            # ---- loads: phase-1-critical 4MB split across the two HWDGE
            # queues (mc on sync, xt-q on scalar). The scalar queue gets
            # ONLY those 8 issues: later ACTIVATEs queue behind DMA issues
            # (strict FIFO), so any extra scalar DMA stalls phase 1.
            rr_sb = wp.tile([P, EC], F32)
            bv_sb = wp.tile([P, E], F32)
            nc.sync.dma_start(rr_sb[:], rr[:, :])
            half = EC // 2
            nc.sync.dma_start(mc_sb[:, 0:half, :], mc_r[:, 0:half, :])
            nc.sync.dma_start(xq_sb[:, 0:half, :], xq_r[:, 0:half, :])
            nc.sync.dma_start(mc_sb[:, half:EC, :], mc_r[:, half:EC, :])
            nc.sync.dma_start(xq_sb[:, half:EC, :], xq_r[:, half:EC, :])
            nc.scalar.dma_start(bv_sb[:], bvb[:, :])
            nc.sync.dma_start(xk_sb[:, :, :], xk_r[:, :, :])
            nc.sync.dma_start(xn_sb[:, :, :], xn_r[:, :, :])
            nc.sync.dma_start(wv_sb[:, :, :], wvt_r[:, :, :])

            # PE warmup: cover the initial DMA latency and release the HAM
            # clock throttle before real matmuls arrive.
            warm_ps = pw.tile([P, P], MM_DT, tag="warm", bufs=2)
            for _ in range(40):
                nc.tensor.transpose(warm_ps[:], ident[:], ident[:])

            QT = kvq.tile([P, EC, SQ], MM_DT)
            attnT = kvq.tile([P, SKC, SQ], MM_DT)
            TT = kvq.tile([P, EC, SQ], MM_DT)

            # ---- phase 1: QM^T[eo, q] = (x_q M)^T + r, in TWO
            # half-contraction passes. Pass A (ec 0-3) needs only the
            # first halves of mc/xq (first 2MB of DMA) and overlaps the
            # arrival of the second 2MB; pass B accumulates ec 4-7 in
            # PSUM and DVE merges the bf16 partial back in. ----
            QTa = kvq.tile([P, EC, SQ], MM_DT)
            H = EC // 2
            for eo in range(EC):
                for g in range(2):
                    gsl = slice(g * NG, (g + 1) * NG)
                    pq = ps.tile([P, NG], F32, tag="mm")
                    for ec in range(H):
                        nc.tensor.matmul(
                            pq[:], mc_sb[:, ec, eo * P:(eo + 1) * P],
                            xq_sb[:, ec, gsl],
                            start=(ec == 0), stop=(ec == H - 1),
                        )
                    nc.scalar.activation(
                        QTa[:, eo, gsl], pq[:],
                        mybir.ActivationFunctionType.Identity,
                        bias=rr_sb[:, eo:eo + 1],
                    )
            for eo in range(EC):
                for g in range(2):
                    gsl = slice(g * NG, (g + 1) * NG)
                    pq = ps.tile([P, NG], F32, tag="mm")
                    for ec in range(H, EC):
                        nc.tensor.matmul(
                            pq[:], mc_sb[:, ec, eo * P:(eo + 1) * P],
                            xq_sb[:, ec, gsl],
                            start=(ec == H), stop=(ec == EC - 1),
                        )
                    nc.vector.tensor_add(
                        QT[:, eo, gsl], pq[:], QTa[:, eo, gsl]
                    )

            # ---- phase 2: scoresT[k, q] = xt_k^T QM^T ; attnT = exp ----
            # DVE accumulates rowsum partials (acc[p,q] = sum_kc attnT)
            acc = kvq.tile([P, SQ], F32)
            for kb in range(SKC):
                xsb, kbi = (xq_sb, kb) if kb < QB else (xk_sb, kb - QB)
                for g in range(2):
                    gsl = slice(g * NG, (g + 1) * NG)
                    p0 = ps.tile([P, NG], F32, tag="mm")
                    for ec in range(EC):
                        nc.tensor.matmul(
                            p0[:], xsb[:, ec, kbi * P:(kbi + 1) * P],
                            QT[:, ec, gsl],
                            start=(ec == 0), stop=(ec == EC - 1),
                        )
                    nc.scalar.activation(
                        attnT[:, kb, gsl], p0[:],
                        mybir.ActivationFunctionType.Exp,
                    )
                    if kb == 0:
                        nc.vector.tensor_copy(acc[:, gsl], attnT[:, 0, gsl])
                    else:
                        nc.vector.tensor_add(
                            acc[:, gsl], acc[:, gsl], attnT[:, kb, gsl]
                        )

            # ---- phase 3: TT[e, q] = x^T-blocks @ attnT ----
            for eb in range(EC):
                for g in range(2):
                    gsl = slice(g * NG, (g + 1) * NG)
                    t0 = ps.tile([P, NG], F32, tag="mm")
                    for kc in range(SKC):
                        nc.tensor.matmul(
                            t0[:], xn_sb[:, kc, eb * P:(eb + 1) * P],
                            attnT[:, kc, gsl],
                            start=(kc == 0), stop=(kc == SKC - 1),
                        )
                    nc.vector.tensor_copy(TT[:, eb, gsl], t0[:])

            # ---- rowsums: rsT[q,qb] via 8 one-column fp32 matmuls ----
            rsT = smalls.tile([P, QB], F32, tag="rsT")
            for qb in range(QB):
                pt = ps.tile([P, 1], F32, tag="mm")
                nc.tensor.matmul(
                    pt[:], acc[:, qb * P:(qb + 1) * P], ones_f[:, 0:1],
                    start=True, stop=True,
                )
                nc.vector.tensor_copy(rsT[:, qb:qb + 1], pt[:])
            recip = smalls.tile([P, QB], F32, tag="recip")
            nc.vector.reciprocal(recip[:], rsT[:])

            # ---- phase 4: out[q, eo] = TT^T Wv^T * recip + bv ----
            for qb in range(QB):
                outt = outp.tile([P, E], MM_DT, tag="out")
                for g in range(2):
                    gsl = slice(g * NG, (g + 1) * NG)
                    o0 = ps.tile([P, NG], F32, tag="mm")
                    for ec in range(EC):
                        nc.tensor.matmul(
                            o0[:], TT[:, ec, qb * P:(qb + 1) * P],
                            wv_sb[:, ec, gsl],
                            start=(ec == 0), stop=(ec == EC - 1),
                        )
                    nc.scalar.activation(
                        outt[:, gsl], o0[:],
                        mybir.ActivationFunctionType.Copy,
                        scale=recip[:, qb:qb + 1],
                    )
                    nc.vector.tensor_add(outt[:, gsl], outt[:, gsl], bv_sb[:, gsl])
                    nc.sync.dma_start(out[qb * P:(qb + 1) * P, gsl], outt[:, gsl])
    nc.finalize()
    return nc


def build_in_maps(x, Wq, bq, Wk, bk, Wv, bv):
    x = np.asarray(x, dtype=np.float32)
    Wq = np.asarray(Wq, np.float32)
    Wk = np.asarray(Wk, np.float32)
    Wv = np.asarray(Wv, np.float32)
    bq = np.asarray(bq, np.float32)
    bv = np.asarray(bv, np.float32)

    # weight-only precompute: fold Wq/Wk and the 1/sqrt(E) scale
    M = (Wq.T @ Wk) * np.float32(INV_SCALE)
    r = (bq @ Wk) * np.float32(INV_SCALE)
    def _chunked(a, n_chunks):
        f = a.shape[1]
        return np.ascontiguousarray(
            a.reshape(n_chunks, P, f).transpose(1, 0, 2).reshape(P, n_chunks * f)
        )

    mcm = _chunked(M.astype(NP_MM), EC)
    wvtm = _chunked(np.ascontiguousarray(Wv.T).astype(NP_MM), EC)
    rrm = np.ascontiguousarray(r.reshape(EC, P).T)      # [P, EC]
    bvb = np.broadcast_to(bv[None, :], (P, E)).copy()

    in_maps = []
    for c in range(8):
        b, h = divmod(c, 2)
        xb = x[b].astype(NP_MM)                         # [S, E]
        if h == 1:
            xb = np.concatenate([xb[SQ:], xb[:SQ]], axis=0)
        xnm = _chunked(xb, SKC)
        xtp = np.ascontiguousarray(xb.T)                # [E, S]
        xqm = _chunked(np.ascontiguousarray(xtp[:, 0:SQ]), EC)
        xkm = _chunked(np.ascontiguousarray(xtp[:, SQ:S]), EC)
        in_maps.append(
            dict(xq2=xqm, xk2=xkm, xn2=xnm, mc2=mcm, wvt2=wvtm, rr=rrm, bvb=bvb)
        )

    return in_maps


def kernel(x, Wq, bq, Wk, bk, Wv, bv):
    if "nc" not in _CACHE:
        _CACHE["nc"] = _build()
    nc = _CACHE["nc"]
    in_maps = build_in_maps(x, Wq, bq, Wk, bk, Wv, bv)
    res = run_bass_kernel_spmd(nc, in_maps, list(range(8)))

    out = np.empty((B, S, E), np.float32)
    for c in range(8):
        b, h = divmod(c, 2)
        out[b, h * SQ:(h + 1) * SQ, :] = np.asarray(res.results[c]["out"], np.float32)
    return out
